# revision 29
# baseline (speedup 1.0000x reference)
"""Trainium2 Bass kernel for the DialogGCN GAT-style message-passing layer.

Math notes (why this is much cheaper than the reference graph):
  Kp    = concat(K, kfeat) @ Wk + bk                    (B,N,D)
  alpha = Q@wden[:D] + Kp@wden[D:] + bden               (B,N)
  w     = softmax(alpha - (1-adj)*1e30, axis=N)
  out   = sum_n w * ((Kp@Wr0)*sm + (Kp@Wr1)*(1-sm))

* softmax is invariant to per-row constants, so the Q term, bden and the
  bk@wden[D:] constant all cancel:  w = softmax_n(X_n . v) masked, where
  X = concat(K, kfeat) and v = Wk @ wden[D:]  (folded on host).
* the output is linear in the weighted sums:
    out = (sum_n w*sm*[X|1]) @ G0 + (sum_n w*(1-sm)*[X|1]) @ G1
  with G0 = [Wk;bk]@Wr0, G1 = [Wk;bk]@Wr1 folded on host (769x512 each).
* tokens with adj=0 get softmax weight exactly 0: they contribute nothing
  to the scores' denominator, U0/U1, or the output. Only ~half the tokens
  are active, so the host gathers active rows per (branch,batch), SORTED by
  the speaker mask sm: first all sm=1 tokens (-> U0), then all sm=0 (-> U1),
  each segment padded to a tile multiple with all-zero rows. Zero rows have
  zero ones-columns, so even though exp(0)=1 they contribute exactly 0 to
  U0/U1 and P -- no mask tensors or mask multiplies are needed on device.

The device streams the gathered X once computing
    s_i = X_i . v ; p_i = exp(s_i) ; U0 = sum_{seg0} p*[X|1] ;
    U1 = sum_{seg1} p*[X|1]
followed by out = (U0@G0 + U1@G1) / P,  P = U0[768] + U1[768].

All X traffic is fp16 (host-cast). K, k1 and the ones columns are packed
host-side into one [NA, 772] array per batch so each 128-token tile needs a
single contiguous DMA. The score dot-product is split across engines:
fused STT on DVE for some tiles, fp16 multiply on DVE (2x packed mode) +
ScalarE activation-accumulate for the rest; the ScalarE exp then writes the
weight column for the tile's segment directly (scores -> exp -> matmul is
the whole chain).

Sharding: pure data parallel over batch B=32 across 8 cores (4 rows each).
"""

import os
import sys

import numpy as np

for _p in ("/opt/trn_rl_repo", "/root/.axon_site/_ro/trn_rl_repo"):
    if os.path.isdir(_p) and _p not in sys.path:
        sys.path.insert(0, _p)

B, N, D, KD = 32, 2048, 512, 256
F = D + KD  # 768
FP = F + 4  # 772: cols 768:772 are ones (P accum); pad keeps 8B alignment
NCORES = 8
BL = B // NCORES  # 4 batch rows per core
CH = 9  # max n-tiles per score->exp->matmul chunk

_BUILD_CACHE = {}
last_results = None  # BassKernelResults of the most recent run (for test.py)


def _build(ntt: int):
    """Trace the Bass program (same NEFF runs SPMD on all 8 cores).

    ntt: token tiles per (branch, batch). The sm=1 segment of each batch row
    lives in partitions [0,64) (quota 64*ntt tokens), the sm=0 segment in
    partitions [64,128) -- so the exp can write each segment's pp weight
    column with one partition-sliced op and no tile-granularity padding.
    """
    import concourse.bass as bass
    import concourse.tile as tile
    from concourse import bacc, mybir
    from concourse.masks import make_identity

    f32 = mybir.dt.float32
    f16 = mybir.dt.float16
    NA = 128 * ntt

    # A = fused STT on DVE (~1030ns/tile), B = DVE fp16 mult at 2x (~550ns)
    # + ScalarE activation-accum (~1080ns). 1:2 ratio balances both engines.
    plan = (["A", "B", "A", "B", "B"] * ((ntt + 4) // 5))[:ntt]
    def _mk_chunks(sz):
        return [list(range(i, min(i + sz, ntt))) for i in range(0, ntt, sz)]

    nc = bacc.Bacc()

    # ---- DRAM I/O ----------------------------------------------------------
    xm_f = nc.dram_tensor("xm_f", [BL, NA, FP], f16, kind="ExternalInput")
    xm_b = nc.dram_tensor("xm_b", [BL, NA, FP], f16, kind="ExternalInput")
    v_f = nc.dram_tensor("v_f", [128, F], f16, kind="ExternalInput")
    v_b = nc.dram_tensor("v_b", [128, F], f16, kind="ExternalInput")
    g0_f = nc.dram_tensor("g0_f", [128, 7, D], f16, kind="ExternalInput")
    g1_f = nc.dram_tensor("g1_f", [128, 7, D], f16, kind="ExternalInput")
    g0_b = nc.dram_tensor("g0_b", [128, 7, D], f16, kind="ExternalInput")
    g1_b = nc.dram_tensor("g1_b", [128, 7, D], f16, kind="ExternalInput")
    out_f = nc.dram_tensor("out_f", [BL, D], f32, kind="ExternalOutput")
    out_b = nc.dram_tensor("out_b", [BL, D], f32, kind="ExternalOutput")

    branches = [
        dict(xm=xm_f, v=v_f, g0=g0_f, g1=g1_f, out=out_f),
        dict(xm=xm_b, v=v_b, g0=g0_b, g1=g1_b, out=out_b),
    ]

    with tile.TileContext(nc) as tc:
        with (
            tc.tile_pool(name="singles", bufs=1) as singles,
            tc.tile_pool(name="xmp", bufs=6) as xmp,
            tc.tile_pool(name="scr", bufs=4) as scr,
            tc.tile_pool(name="small", bufs=4) as small,
            tc.tile_pool(name="uallp", bufs=2) as uallp,
            tc.tile_pool(name="uallTp", bufs=2) as uallTp,
            tc.tile_pool(name="finp", bufs=2) as finp,
            tc.tile_pool(name="psU_K", bufs=1, space="PSUM") as psU_K,
            tc.tile_pool(name="psU_1", bufs=1, space="PSUM") as psU_1,
            tc.tile_pool(name="psTr", bufs=2, space="PSUM") as psTr,
            tc.tile_pool(name="psOut", bufs=2, space="PSUM") as psOut,
        ):
            # ---- one-time setup -------------------------------------------
            ident = singles.tile([128, 128], f32)
            make_identity(nc, ident)

            units = [(bi, b) for b in range(BL) for bi in range(2)]

            st = []
            for br in branches:
                d = {}
                vb = singles.tile([128, F], f16, tag=f"vb_{br['v'].name}")
                nc.gpsimd.dma_start(out=vb, in_=br["v"][:, :])
                d["vb"] = vb
                d["psK"] = psU_K.tile([8, D], f32, name=f"psK_{br['out'].name}")
                d["ps1"] = psU_1.tile([8, FP - D], f32, name=f"ps1_{br['out'].name}")
                st.append(d)

            # issue the first units' X DMAs before the (large, only needed at
            # the end) G loads so compute starts as early as possible
            xms = {}
            for ui in range(2):
                bi, b = units[ui]
                xm = xmp.tile([128, ntt, FP], f16, tag="xm", name=f"xm_pre{ui}")
                xsrc = branches[bi]["xm"][b].rearrange("(p n) d -> p n d", n=ntt)
                # partition-quarter pieces land on different DMA queues, so
                # the first tile arrives ~4x sooner than via a single queue
                for q in range(4):
                    nc.gpsimd.dma_start(
                        out=xm[q * 32 : (q + 1) * 32],
                        in_=xsrc[q * 32 : (q + 1) * 32],
                    )
                xms[ui] = xm

            # ---- streaming: interleave branches so finishing overlaps -----
            for ui, (bi, b) in enumerate(units):
                br, sd = branches[bi], st[bi]
                if ui in xms:
                    xm = xms[ui]
                else:
                    xm = xmp.tile([128, ntt, FP], f16, tag="xm")
                    nc.gpsimd.dma_start(
                        out=xm, in_=br["xm"][b].rearrange("(p n) d -> p n d", n=ntt)
                    )
                if ui == 3:
                    # G matrices are only needed at finishing; issuing them
                    # here keeps them out of the early X stream
                    for br2, d2 in zip(branches, st):
                        for gn in ("g0", "g1"):
                            gs = singles.tile(
                                [128, 7, D], f16, tag=f"{gn}_{br2['out'].name}"
                            )
                            nc.gpsimd.dma_start(out=gs, in_=br2[gn][:, :, :])
                            d2[gn] = gs

                sB = small.tile([128, ntt], f32, tag="sB")
                pp = small.tile([128, 8, ntt], f16, tag="pp")
                nc.gpsimd.memset(pp, 0.0)
                # separate dummy-out tiles per engine: sharing one creates
                # cross-engine WAR chains that serialize the whole pipeline
                dumpA = small.tile([128, F], f16, tag="dumpA")
                dumpS = small.tile([128, F], f16, tag="dumpS")

                # engine queues stay free of head-of-line stalls: DVE runs
                # only score ops; ScalarE accumulates + exps straight into
                # the pp weight column; PE runs the chunk matmuls.
                # Finer chunks for the last units shorten the end-of-kernel
                # dependency chain into the finishes.
                for ch in _mk_chunks(CH if ui < 6 else (ntt + 1) // 2):
                    lo, hi = ch[0], ch[-1] + 1
                    for n in ch:
                        # score dot s[:,n] = xm[:,n,0:768] . v
                        if plan[n] == "A":
                            nc.vector.scalar_tensor_tensor(
                                out=dumpA,
                                in0=xm[:, n, 0:F],
                                scalar=0.0,
                                in1=sd["vb"],
                                op0=mybir.AluOpType.bypass,
                                op1=mybir.AluOpType.mult,
                                accum_out=sB[:, n : n + 1],
                            )
                        else:
                            prod = scr.tile([128, F], f16, tag="prodB")
                            nc.vector.tensor_mul(prod, xm[:, n, 0:F], sd["vb"])
                            nc.scalar.activation(
                                out=dumpS,
                                in_=prod,
                                func=mybir.ActivationFunctionType.Copy,
                                accum_out=sB[:, n : n + 1],
                            )
                    # sm=1 tokens (partitions 0:64) weight column b -> U0,
                    # sm=0 tokens (partitions 64:128) column 4+b -> U1
                    nc.scalar.activation(
                        out=pp[0:64, b, lo:hi],
                        in_=sB[0:64, lo:hi],
                        func=mybir.ActivationFunctionType.Exp,
                    )
                    nc.scalar.activation(
                        out=pp[64:128, 4 + b, lo:hi],
                        in_=sB[64:128, lo:hi],
                        func=mybir.ActivationFunctionType.Exp,
                    )
                    for n in ch:
                        first = b == 0 and n == 0
                        last = b == BL - 1 and n == ntt - 1
                        nc.tensor.matmul(
                            sd["psK"], pp[:, :, n], xm[:, n, 0:D], start=first, stop=last
                        )
                        nc.tensor.matmul(
                            sd["ps1"], pp[:, :, n], xm[:, n, D:FP], start=first, stop=last
                        )

                # ---- finishing: out = (U0@G0 + U1@G1) / P ------------------
                if b == BL - 1:
                    uall = uallp.tile([8, F + 1], f32)
                    nc.scalar.activation(
                        out=uall[:, 0:D], in_=sd["psK"],
                        func=mybir.ActivationFunctionType.Copy,
                    )
                    nc.vector.tensor_copy(uall[:, D : F + 1], sd["ps1"][:, 0 : KD + 1])

                    uallT = uallTp.tile([128, 7, 8], f16)
                    for k in range(6):
                        trp = psTr.tile([128, 8], f32, tag="trp")
                        nc.tensor.transpose(
                            trp, uall[:, k * 128 : (k + 1) * 128], ident[0:8, 0:8]
                        )
                        nc.scalar.activation(
                            out=uallT[:, k, :], in_=trp,
                            func=mybir.ActivationFunctionType.Copy,
                        )
                    trp = psTr.tile([128, 8], f32, tag="trp")
                    nc.tensor.transpose(trp[0:1, :], uall[:, F : F + 1], ident[0:8, 0:8])
                    nc.scalar.activation(
                        out=uallT[0:1, 6, :], in_=trp[0:1, :],
                        func=mybir.ActivationFunctionType.Copy,
                    )

                    # G-projection matmuls BEFORE the reciprocal chain: the
                    # chain's tiny transpose would otherwise sit in the PE
                    # queue ahead of these and stall them on a DVE round-trip
                    po = psOut.tile([4, D], f32)
                    for k in range(6):
                        nc.tensor.matmul(
                            po, uallT[:, k, 0:4], sd["g0"][:, k, :],
                            start=(k == 0), stop=False,
                        )
                    nc.tensor.matmul(
                        po, uallT[0:1, 6, 0:4], sd["g0"][0:1, 6, :],
                        start=False, stop=False,
                    )
                    for k in range(6):
                        nc.tensor.matmul(
                            po, uallT[:, k, 4:8], sd["g1"][:, k, :],
                            start=False, stop=False,
                        )
                    nc.tensor.matmul(
                        po, uallT[0:1, 6, 4:8], sd["g1"][0:1, 6, :],
                        start=False, stop=True,
                    )

                    # P(b) = U0_768(b) + U1_768(b); rp = 1/P transposed to [4,1]
                    prow = finp.tile([1, 8], f32, tag="prow")
                    nc.vector.tensor_copy(prow, trp[0:1, :])
                    padd = finp.tile([1, 4], f32, tag="padd")
                    nc.vector.tensor_add(padd, prow[0:1, 0:4], prow[0:1, 4:8])
                    rrow = finp.tile([1, 4], f32, tag="rrow")
                    nc.vector.reciprocal(rrow, padd)
                    trp2 = psTr.tile([4, 1], f32, tag="trp")
                    nc.tensor.transpose(trp2, rrow, ident[0:1, 0:1])
                    rp = finp.tile([4, 1], f32, tag="rp")
                    nc.vector.tensor_copy(rp, trp2)

                    osb = finp.tile([4, D], f32, tag="osb")
                    nc.vector.tensor_scalar_mul(out=osb, in0=po, scalar1=rp)
                    nc.sync.dma_start(out=br["out"][:, :], in_=osb)

    nc.compile()
    return nc


def _get_nc(ntt: int):
    if ntt not in _BUILD_CACHE:
        _BUILD_CACHE[ntt] = _build(ntt)
    return _BUILD_CACHE[ntt]


def _gather_idx(adj, sm):
    """Per batch row: active-token indices split by speaker mask."""
    idx0, idx1 = [], []
    for b in range(B):
        act = adj[b] != 0
        idx0.append(np.nonzero(act & (sm[b] != 0))[0])
        idx1.append(np.nonzero(act & (sm[b] == 0))[0])
    longest = max(max(len(ix) for ix in idx0), max(len(ix) for ix in idx1))
    return idx0, idx1, max(1, (longest + 63) // 64)


def _gather_pack(Kv, k1, idx0, idx1, ntt):
    """Pack [K | k1 | ones] fp16: sm=1 tokens fill partitions [0,64), sm=0
    tokens partitions [64,128), each quota 64*ntt tokens, zero-padded.

    The device views the [NA, FP] array as [128, ntt, FP] with flat token
    index p*ntt + n, so segment s occupies flat rows [s*64*ntt, (s+1)*64*ntt).
    """
    quota = 64 * ntt
    Xg = np.zeros((B, 2 * quota, FP), np.float16)
    for b in range(B):
        for seg, ix in enumerate((idx0[b], idx1[b])):
            off = seg * quota
            na = len(ix)
            Xg[b, off : off + na, :D] = Kv[b, ix]
            Xg[b, off : off + na, D:F] = k1[b, ix]
            Xg[b, off : off + na, F:] = 1.0
    return Xg


def _pack_g(A64, Wr):
    """[Wk;bk]@Wr folded to the on-chip [128, 7, D] chunk layout, fp16."""
    G = (A64 @ Wr.astype(np.float64)).astype(np.float16)  # (769, 512)
    out = np.zeros((128, 7, D), np.float16)
    out[:, 0:6, :] = G[0:F].reshape(6, 128, D).transpose(1, 0, 2)
    out[0, 6, :] = G[F]
    return out


def kernel(**inputs) -> tuple:
    global last_results
    from concourse.bass_utils import run_bass_kernel_spmd

    f32 = np.float32
    K = np.asarray(inputs["K"], dtype=f32)
    front_k1 = np.asarray(inputs["front_k1"], dtype=f32)
    back_K = np.asarray(inputs["back_K"], dtype=f32)
    back_k2 = np.asarray(inputs["back_k2"], dtype=f32)
    Wfk = np.asarray(inputs["Wfk"], dtype=f32)
    bfk = np.asarray(inputs["bfk"], dtype=f32)
    Wbk = np.asarray(inputs["Wbk"], dtype=f32)
    bbk = np.asarray(inputs["bbk"], dtype=f32)
    Wr0 = np.asarray(inputs["Wr0"], dtype=f32)
    Wr1 = np.asarray(inputs["Wr1"], dtype=f32)
    wf_den = np.asarray(inputs["wf_den"], dtype=f32)
    wb_den = np.asarray(inputs["wb_den"], dtype=f32)
    adj_f = np.asarray(inputs["front_sdj_den"], dtype=np.int32)
    sm_f = np.asarray(inputs["front_s_mask"], dtype=np.int32)
    adj_b = np.asarray(inputs["back_sdj_den"], dtype=np.int32)
    sm_b = np.asarray(inputs["back_s_mask"], dtype=np.int32)
    i = int(np.asarray(inputs["i"]))
    num_utter = int(np.asarray(inputs["num_utter"]))

    # host-folded weights (parameter preprocessing only)
    v_f = np.ascontiguousarray(
        np.broadcast_to(
            (Wfk.astype(np.float64) @ wf_den[D:].astype(np.float64)).astype(np.float16),
            (128, F),
        )
    )
    v_b = np.ascontiguousarray(
        np.broadcast_to(
            (Wbk.astype(np.float64) @ wb_den[D:].astype(np.float64)).astype(np.float16),
            (128, F),
        )
    )
    A_f = np.vstack([Wfk, bfk[None, :]]).astype(np.float64)
    A_b = np.vstack([Wbk, bbk[None, :]]).astype(np.float64)
    G0_f = _pack_g(A_f, Wr0)
    G1_f = _pack_g(A_f, Wr1)
    G0_b = _pack_g(A_b, Wr0)
    G1_b = _pack_g(A_b, Wr1)

    # input marshaling: active-token gather sorted by sm + fp16 cast/packing
    i0f, i1f, ntf = _gather_idx(adj_f, sm_f)
    i0b, i1b, ntb = _gather_idx(adj_b, sm_b)
    ntt = max(ntf, ntb)
    X_f = _gather_pack(K, front_k1, i0f, i1f, ntt)
    X_b = _gather_pack(back_K, back_k2, i0b, i1b, ntt)

    nc = _get_nc(ntt)

    in_maps = []
    for c in range(NCORES):
        s = slice(c * BL, (c + 1) * BL)
        in_maps.append(
            {
                "xm_f": X_f[s],
                "xm_b": X_b[s],
                "v_f": v_f,
                "v_b": v_b,
                "g0_f": G0_f,
                "g1_f": G1_f,
                "g0_b": G0_b,
                "g1_b": G1_b,
            }
        )

    trace = os.environ.get("KERNEL_TRACE", "0") == "1"
    res = run_bass_kernel_spmd(nc, in_maps, core_ids=list(range(NCORES)), trace=trace)
    last_results = res

    front = np.concatenate([r["out_f"] for r in res.results], axis=0)
    back = np.concatenate([r["out_b"] for r in res.results], axis=0)
    if i == 0:
        front = np.zeros((B, D), dtype=f32)
    if i == num_utter - 1:
        back = np.zeros((B, D), dtype=f32)
    return (front, back)
